# revision 18
# baseline (speedup 1.0000x reference)
"""MetapathAggrLayer Trainium2 kernel — v10 (bf16 store; depth-3 combine).

Per node n: e_m = leakyrelu(x[m,n,:].a), w = softmax(e), out = sum_m w_m x[m,n,:].
Data-parallel over N across 8 NeuronCores; nodes-on-partitions layout.

Key TRN2 constraint discovered on HW: DVE 2-port ops (any tensor_tensor /
custom 2-input op) and GpSimd arbitrate an EXCLUSIVE lock on a shared SBUF
port pair — whichever starts first fully blocks the other for the whole
instruction. DVE is ~95% 2-port here, so GpSimd compute extends the period
1:1 and is useless: all products/adds go on DVE, the per-t scale loop on
Scalar (private port, 331ns per [128,64] op), nothing on GpSimd, and the
store is HWDGE fp32 (no SWDGE descriptor generation on GpSimd either).

Macro-tile = 4096 nodes (128 partitions x T=32), software-pipelined:

  iter i   Sync : load(i) [merged fp32 [128,(m t f)]], store(i-3)
           DVE  : scan_a(i), lrelu_a(i), scan_b(i), lrelu_b(i),
                  a01(i-3), a23(i-3), acc(i-3) [fp32 out],
                  sred(i), recip(i), w(i), t0[19:32](i), t1(i), t3(i)
           ACT  : t2[0:32](i-1), exp(i), t0[0:19](i-1)

Scores: two merged multiply+prefix-scan custom DVE ops (metapath pairs)
sharing one half-size P buffer; segment sums via boundary differences fused
with leakyrelu; softmax sum via a single strided tensor_reduce.
"""

import sys

sys.path.insert(0, "/opt/trn_rl_repo")

import numpy as np

import concourse.bacc as bacc
import concourse.mybir as mybir
from concourse import bass_utils, dve_ops
from concourse.dve_spec import Spec, Src0, Src1, C0, scan, maxx, AluOp, lower, _has_src1
from concourse.dve_uop import DveOpSpec
from concourse.tile import TileContext

ALPHA = 0.2
NMETA = 4
F = 64
N_FULL = 1_000_000
N_CORES = 8
T = 32                     # nodes per partition per macro-tile
NODES_PER_MACRO = 128 * T  # 4096
MACROS_PER_CORE = 31
NC_NODES = MACROS_PER_CORE * NODES_PER_MACRO  # 126_976
N_PAD = N_CORES * NC_NODES                    # 1_015_808
NSEG = NMETA * T           # score segments per partition per macro
NM = T * F                 # per-metapath free elems
NALL = NMETA * NM          # merged free elems
NH = NALL // 2             # half (one metapath pair)
T0_ACT = 22                # t0 chunks on Scalar; rest on Vector

_CACHE = {}


def _register_op(name, spec, subdim=False):
    if name in dve_ops._SUB_OPCODE_FOR_NAME:
        return next(o for o in dve_ops.OPS if o.name == name)
    row = dve_ops._CUSTOM_DVE_ROW_BASE + len(dve_ops.OPS)
    assert row < 0x20
    shas = {}
    for ver in ("v3", "v4"):
        s = DveOpSpec(name=name, opcode=row, uops=lower(spec, ver=ver),
                      rd1_en=_has_src1(spec))
        shas[ver] = s.sha(ver)
    op = dve_ops.DveOp(name, spec, subdim, shas)
    dve_ops.OPS.append(op)
    dve_ops.CUSTOM_DVE_SPECS[name] = spec
    dve_ops._SUB_OPCODE_FOR_NAME[name] = row
    return op


def _get_ops():
    scan_mul = _register_op(
        "MPA_SCAN_MUL",
        Spec(
            body=scan(AluOp.ADD, Src0 * Src1),
            reference=lambda in0, in1, s0, s1: np.cumsum(
                (in0.astype(np.float32) * in1.astype(np.float32)), axis=-1
            ),
        ),
    )
    ext_lrelu = _register_op(
        "MPA_EXT_LRELU",
        Spec(
            body=(lambda d: maxx(d, d * C0))(Src0 - Src1),
            reference=lambda in0, in1, s0, s1: np.maximum(in0 - in1, (in0 - in1) * s0),
        ),
    )
    return scan_mul, ext_lrelu


def _build_kernel():
    scan_mul, ext_lrelu = _get_ops()

    nc = bacc.Bacc("TRN2", target_bir_lowering=False, debug=False)
    f32 = mybir.dt.float32
    bf16 = mybir.dt.bfloat16

    x_in = nc.dram_tensor("input", (NMETA, NC_NODES, F), f32, kind="ExternalInput").ap()
    a_rep_in = nc.dram_tensor("a_rep", (128, F), f32, kind="ExternalInput").ap()
    out = nc.dram_tensor("out", (NC_NODES, F), bf16, kind="ExternalOutput").ap()

    mult = mybir.AluOpType.mult
    add = mybir.AluOpType.add
    M = MACROS_PER_CORE

    with TileContext(nc) as tc:
        with tc.tile_pool(name="const", bufs=1) as cpool, \
             tc.tile_pool(name="xp", bufs=3) as xpool, \
             tc.tile_pool(name="prod", bufs=3) as prpool, \
             tc.tile_pool(name="comb", bufs=2) as copool, \
             tc.tile_pool(name="small", bufs=3) as spool:
            a_rep = cpool.tile([128, F], f32)
            nc.sync.dma_start(out=a_rep[:, :], in_=a_rep_in)
            a_bc = a_rep[:, :].rearrange("p (o f) -> p o f", o=1).broadcast_to(
                [128, NH // F, F])
            P = cpool.tile([128, NH + 1], f32)
            nc.gpsimd.memset(P[:, 0:1], 0.0)

            tiles = {}

            def wb(d, m, t0=0, t1=T):
                return d["w"][:, m * T + t0:m * T + t1].rearrange(
                    "p (t o) -> p t o", o=1).broadcast_to([128, t1 - t0, F])

            def x3(d, m, t0=0, t1=T):
                return d["xm"][:, m * NM + t0 * F:m * NM + t1 * F].rearrange(
                    "p (t f) -> p t f", f=F)

            for v in range(M + 3):
                # ---- stage A: load + scores (two half scans share P)
                if v < M:
                    lo = v * NODES_PER_MACRO
                    hi = lo + NODES_PER_MACRO
                    d = tiles[v] = {"lo": lo, "hi": hi}
                    d["xm"] = xpool.tile([128, NALL], f32, tag="x", name="xm")
                    src = x_in[:, lo:hi, :].rearrange("m (p t) f -> p m t f", p=128)
                    dst4 = d["xm"][:, :].rearrange("p (m t f) -> p m t f",
                                                   m=NMETA, f=F)
                    nc.sync.dma_start(out=dst4, in_=src)

                    d["e"] = spool.tile([128, NSEG], f32, tag="e", name="e")
                    nseg_k = NH // F
                    for k in range(2):
                        nc.vector._custom_dve(
                            scan_mul, out=P[:, 1:NH + 1],
                            in0=d["xm"][:, k * NH:(k + 1) * NH], in1=a_bc,
                        )
                        p_hi = P[:, 1:NH + 1].rearrange(
                            "p (s f) -> p s f", f=F)[:, :, F - 1:F]
                        p_lo = P[:, 0:NH].rearrange(
                            "p (s f) -> p s f", f=F)[:, :, 0:1]
                        nc.vector._custom_dve(
                            ext_lrelu,
                            out=d["e"][:, k * nseg_k:(k + 1) * nseg_k],
                            in0=p_hi, in1=p_lo, s0=ALPHA,
                        )

                # ---- ACT t2 share (iter v-1) first in the ACT queue
                if 0 <= v - 1 < M:
                    db = tiles[v - 1]
                    db["t2"] = prpool.tile([128, NM], bf16, tag="t2", name="t2")
                    for t in range(T):
                        fs = t * F
                        nc.scalar.mul(db["t2"][:, fs:fs + F],
                                      db["xm"][:, 2 * NM + fs:2 * NM + fs + F],
                                      db["w"][:, 2 * T + t:2 * T + t + 1])

                # ---- combine (iter v-3) on Vector + HWDGE store
                if 0 <= v - 3 < M:
                    dc = tiles[v - 3]
                    dc["a01"] = copool.tile([128, NM], bf16, tag="a01", name="a01")
                    dc["a23"] = copool.tile([128, NM], bf16, tag="a23", name="a23")
                    dc["acc"] = copool.tile([128, NM], bf16, tag="acc", name="acc")
                    nc.vector.tensor_tensor(out=dc["a01"][:, :], in0=dc["t0"][:, :],
                                            in1=dc["t1"][:, :], op=add)
                    nc.vector.tensor_tensor(out=dc["a23"][:, :], in0=dc["t2"][:, :],
                                            in1=dc["t3"][:, :], op=add)
                    nc.vector.tensor_tensor(out=dc["acc"][:, :], in0=dc["a01"][:, :],
                                            in1=dc["a23"][:, :], op=add)
                    dst = out[dc["lo"]:dc["hi"], :].rearrange(
                        "(p t) f -> p (t f)", p=128)
                    nc.sync.dma_start(out=dst, in_=dc["acc"][:, :])
                    del tiles[v - 3]

                # ---- stage A cont.: softmax chain + DVE product shares + exp
                if v < M:
                    d = tiles[v]
                    d["u"] = spool.tile([128, NSEG], f32, tag="u", name="u")
                    nc.scalar.activation(d["u"][:, :], d["e"][:, :],
                                         mybir.ActivationFunctionType.Exp)
                    # sum over metapaths: strided reduce of [128, T, NMETA]
                    d["s"] = spool.tile([128, T], f32, tag="s", name="s")
                    u_tm = d["u"][:, :].rearrange("p (m t) -> p t m", m=NMETA)
                    nc.vector.tensor_reduce(out=d["s"][:, :], in_=u_tm,
                                            axis=mybir.AxisListType.X, op=add)
                    d["r"] = spool.tile([128, T], f32, tag="r", name="r")
                    nc.vector.reciprocal(d["r"][:, :], d["s"][:, :])
                    d["w"] = spool.tile([128, NSEG], f32, tag="w", name="w")
                    r_bc = d["r"][:, :].rearrange(
                        "p (o t) -> p o t", o=1).broadcast_to([128, NMETA, T])
                    nc.vector.tensor_tensor(
                        out=d["w"][:, :].rearrange("p (m t) -> p m t", m=NMETA),
                        in0=d["u"][:, :].rearrange("p (m t) -> p m t", m=NMETA),
                        in1=r_bc, op=mult)

                    d["t0"] = prpool.tile([128, NM], bf16, tag="t0", name="t0")
                    d["t1"] = prpool.tile([128, NM], bf16, tag="t1", name="t1")
                    d["t3"] = prpool.tile([128, NM], bf16, tag="t3", name="t3")
                    nc.vector.tensor_tensor(
                        out=d["t0"][:, T0_ACT * F:].rearrange(
                            "p (t f) -> p t f", f=F),
                        in0=x3(d, 0, T0_ACT, T), in1=wb(d, 0, T0_ACT, T), op=mult)
                    nc.vector.tensor_tensor(
                        out=d["t1"][:, :].rearrange("p (t f) -> p t f", f=F),
                        in0=x3(d, 1), in1=wb(d, 1), op=mult)
                    nc.vector.tensor_tensor(
                        out=d["t3"][:, :].rearrange("p (t f) -> p t f", f=F),
                        in0=x3(d, 3), in1=wb(d, 3), op=mult)

                # ---- ACT t0 head (iter v-1)
                if 0 <= v - 1 < M:
                    db = tiles[v - 1]
                    for t in range(T0_ACT):
                        fs = t * F
                        nc.scalar.mul(db["t0"][:, fs:fs + F], db["xm"][:, fs:fs + F],
                                      db["w"][:, t:t + 1])

    nc.compile()
    return nc


def kernel(input, a, _trace=False):
    input = np.ascontiguousarray(np.asarray(input, dtype=np.float32))
    a = np.asarray(a, dtype=np.float32).reshape(F)

    if "nc" not in _CACHE:
        _CACHE["nc"] = _build_kernel()
    nc = _CACHE["nc"]

    pad = N_PAD - input.shape[1]
    xp = np.concatenate(
        [input, np.zeros((NMETA, pad, F), np.float32)], axis=1
    ) if pad else input

    a_rep = np.tile(a[None, :], (128, 1)).astype(np.float32)

    in_maps = []
    for c in range(N_CORES):
        sl = xp[:, c * NC_NODES:(c + 1) * NC_NODES, :]
        in_maps.append({"input": np.ascontiguousarray(sl), "a_rep": a_rep})

    res = bass_utils.run_bass_kernel_spmd(
        nc, in_maps, core_ids=list(range(N_CORES)), trace=_trace
    )
    outs = [np.asarray(res.results[c]["out"]).astype(np.float32)
            for c in range(N_CORES)]
    full = np.concatenate(outs, axis=0)[:N_FULL]
    if _trace:
        return full, res
    return full
